# revision 81
# baseline (speedup 1.0000x reference)
"""Trainium2 Bass kernel for a pre-norm transformer block (dense_transformer).

Computation (per reference):
    x = x + Attn(LN1(x));  x = x + MLP(LN2(x))
with causal multi-head attention (H=16 heads, D=64) and a 4E ReLU MLP.

Sharding: 8 cores = 2 batches x 4 query-blocks of 512 tokens (no
collectives).  Context is rotated so the query block sits in slots
[T-512, T); padded slots are masked via a -1e9 additive bias folded into
the softmax exp bias (colmask); the causal diagonal band is accumulated
into the score PSUM as an identity-weight matmul of a {0,-1e9} tile.

Speed strategy vs the bf16 baseline (675950ns -> ~376700ns):
  * fp8e4m3 DoubleRow matmuls (256-deep contraction at 0.5 cycles/col) for
    the Q/K/V projections, attn@V (t-tile pairs packed along the free
    axis), and the output projection.  The MLP stays bf16 (fp8 there
    breaks the 2e-2 gate; attention-wide fp8 measures ~8e-3).
  * V carries a ones column per head, so row 64 of each attn@V psum
    accumulates the softmax row-sums for free (no row-sum matmuls).
  * exp writes fp8 probabilities directly, shifted by exp(-3) (folded into
    colmask, cancelled exactly by the row-sum normalization) to keep
    values inside fp8 range.  Attention runs as 8 two-head groups with
    double-buffered score psums; exp on ACT is the pacing engine, and the
    next groups' K projections + second-half V projections fill the PE
    slack.  Per-group normalization: DVE reciprocal of psum row 64,
    GPSIMD partition_broadcast, multiply straight out of PSUM into fp8.
  * LN gains/biases folded into the weights host-side (W' = diag(g)W,
    bias rows b@W), so LN emits raw (x-mu)*rstd in 2 DVE/GPSIMD ops per
    tile; LN row stats broadcast via GPSIMD partition_broadcast instead
    of ones-matmuls.
  * x streamed as bf16 in column-chunk order (LN1 starts after 1/4 of x)
    + a small fp32 slice for the residual; W1 partially prefetched into
    the SBUF freed by x during attention; W2 streamed in exactly the
    column slices each MLP2 pass needs.
"""

from dataclasses import dataclass

import numpy as np
import ml_dtypes

import concourse.bass as bass  # noqa: F401
import concourse.mybir as mybir
import concourse.tile as tile
from concourse import bacc
from concourse import bass_utils

F32 = mybir.dt.float32
BF16 = mybir.dt.bfloat16
FP8 = mybir.dt.float8e4
AF = mybir.ActivationFunctionType
OP = mybir.AluOpType
PM = mybir.MatmulPerfMode
NPBF16 = ml_dtypes.bfloat16
NPF8 = ml_dtypes.float8_e4m3

P = 128
NEG = -1.0e9
ESHIFT = -3.0  # exp(s + ESHIFT): fp8 headroom; cancelled by normalization


@dataclass(frozen=True)
class Cfg:
    B: int = 2
    T: int = 2048
    E: int = 1024
    H: int = 16
    D: int = 64
    NC: int = 8
    eps: float = 1e-5

    @property
    def CPB(self):
        return self.NC // self.B

    @property
    def Tq(self):
        return self.T // self.CPB

    @property
    def KE(self):
        return self.E // P

    @property
    def TK(self):
        return self.T // P

    @property
    def HP(self):
        return self.H // 2

    @property
    def NB(self):
        return self.Tq // P

    @property
    def F(self):
        return 4 * self.E

    @property
    def KF(self):
        return self.F // P

    @property
    def TCH(self):
        return min(512, self.T)

    @property
    def NQC(self):
        return self.T // self.TCH

    def check(self):
        assert self.D == 64 and self.E == self.H * self.D
        assert self.Tq <= 512 and self.Tq % P == 0
        assert self.T % self.TCH == 0 and self.E % P == 0 and self.F % P == 0
        assert self.NC % self.B == 0 and self.H % 4 == 0 and self.KE % 2 == 0


CFG = Cfg()


class Pools:
    """Tile pools with explicit open/close (LIFO per side, per space)."""

    def __init__(self, tc, prefix=""):
        self.tc = tc
        self.prefix = prefix
        self.live = {}

    def open(self, key, bufs, space=None, side=None):
        kw = dict(name=self.prefix + key, bufs=bufs)
        if space:
            kw["space"] = space
        if side:
            kw["side"] = side
        cm = self.tc.tile_pool(**kw)
        pool = cm.__enter__()
        self.live[key] = cm
        return pool

    def close(self, *keys):
        for key in keys:
            self.live.pop(key).__exit__(None, None, None)

    def close_all(self):
        for key in reversed(list(self.live)):
            self.close(key)


def _emit(tc, c: Cfg, d, reps: int = 1):
    for _rep in range(reps):
        _emit_one(tc, c, d, _rep)


def _emit_one(tc, c: Cfg, d, rep: int):
    nc = tc.nc
    E, T, Tq, H, D = c.E, c.T, c.Tq, c.H, c.D
    KE, TK, HP, NB, KF = c.KE, c.TK, c.HP, c.NB, c.KF
    TCH, NQC = c.TCH, c.NQC
    CP = KE // 2  # xn chunk-pairs (DoubleRow contraction steps)
    SCL = 1.0 / float(np.sqrt(D))

    pp = Pools(tc, prefix=f"r{rep}_")

    # ---------------- constants (whole-kernel lifetime) --------------------
    const = pp.open("const", 1)
    ones_bf = const.tile([P, 1], BF16, name="ones_bf")
    nc.vector.memset(ones_bf[:], 1.0)
    ones1b = const.tile([1, P], BF16, name="ones1b")
    nc.vector.memset(ones1b[:], 1.0)
    warmt = const.tile([P, 512], BF16, name="warmt")
    nc.vector.memset(warmt[:], 0.5)
    # const DMAs are issued on the Pool DGE queue so they never delay the
    # latency-critical x loads on the SP queue
    colmask = const.tile([P, TK], F32, name="colmask")
    nc.gpsimd.dma_start(colmask[:], d["colmask"])
    ident = const.tile([P, P], BF16, name="ident")
    nc.gpsimd.dma_start(ident[:], d["ident"])
    negband = const.tile([P, NB, Tq], BF16, name="negband")
    nc.gpsimd.dma_start(negband[:], d["negband"])
    gbt = {}
    for nm, cols in [
        ("qb", KE), ("kb", KE), ("boc", KE), ("mb1", KF), ("mb2", KE),
    ]:
        gbt[nm] = const.tile([P, cols], F32, name=nm + "_t")
        nc.gpsimd.dma_start(gbt[nm][:], d[nm])
    vbrow = const.tile([1, E], BF16, name="vbrow")
    nc.gpsimd.dma_start(vbrow[:], d["vbrow"])
    vbT = const.tile([P, H, D], BF16, name="vbT")

    # ---------------- warmup (PE p-state ramp) -----------------------------
    ps_wm = pp.open("warm_ps", 1, "PSUM")
    wmp = ps_wm.tile([1, 512], F32, name="wmp")
    for _w in range(12):
        nc.tensor.matmul(wmp[:], ones_bf[:], warmt[:], start=True, stop=True)
    pp.close("warm_ps")

    # vbT = ones ⊗ vbrow (broadcast V bias over partitions)
    ps_vb = pp.open("vb_ps", 1, "PSUM")
    for g2 in range(E // 512):
        vps = ps_vb.tile([P, 8, D], F32, name="vbps")
        nc.tensor.matmul(
            vps[:], ones1b[:], vbrow[:, g2 * 512 : (g2 + 1) * 512],
            start=True, stop=True,
        )
        nc.vector.tensor_copy(vbT[:, g2 * 8 : (g2 + 1) * 8, :], vps[:])
    pp.close("vb_ps")

    # ---------------- long-lived activation tiles --------------------------
    # Left-stack order is chosen so closes nest LIFO: pools closing sooner
    # sit higher.  xtp sits above the attention-lifetime tiles; wq8 goes on
    # the right stack (it closes at the end of phase 0, before w1 opens).
    p_xq = pp.open("xqp", 1)
    xq32 = [p_xq.tile([P, Tq], F32, name=f"xq{e}") for e in range(KE)]
    p_wo8 = pp.open("wo8p", 1)
    wo8s = p_wo8.tile([P, KE, CP, 2, P], FP8, name="wo8s")
    p_xn8 = pp.open("xn8p", 1)
    xn8 = [p_xn8.tile([P, 2, T], FP8, name=f"xn8_{cp}") for cp in range(CP)]
    p_qt = pp.open("qtp", 1)
    qt = [p_qt.tile([P, Tq], BF16, name=f"qt{j}") for j in range(HP)]
    p_kt = pp.open("ktp", 1)
    kt = [p_kt.tile([P, T], BF16, name=f"kt{j}") for j in range(HP)]
    p_wk8 = pp.open("wk8p", 1)
    wk8s = p_wk8.tile([P, HP, CP, 2, P], FP8, name="wk8s")
    p_wq8 = pp.open("wq8p", 1)
    wq8s = p_wq8.tile([P, HP, CP, 2, P], FP8, name="wq8s")

    # ---------------- input loads ------------------------------------------
    p_xt = pp.open("xtp", 1)
    xtb = [p_xt.tile([P, T], BF16, name=f"xtb{e}") for e in range(KE)]
    # column-chunk-major loads so LN1 stats of chunk 0 start after ~1/4 of
    # x; alternate between two DGE queues to halve the per-DMA issue cost
    for cc in range(NQC):
        cs = slice(cc * TCH, (cc + 1) * TCH)
        for e in range(KE):
            nc.sync.dma_start(xtb[e][:, cs], d["xtb"][e * P : (e + 1) * P, cs])
    nc.sync.dma_start(wq8s[:], d["wq8"])
    nc.sync.dma_start(wk8s[:], d["wk8"])

    # ======================================================================
    # Phase 0: LN1 (feature-major, per-column stats), emits fp8 xn8 in
    # chunk-paired DoubleRow layout; QKV bundles interleaved per col-chunk.
    # ======================================================================
    p_tmp = pp.open("ln_tmp", 2)
    p_rows = pp.open("ln_rows", 1)
    p_stg = pp.open("ln_stg", 2)
    ps_qkv = pp.open("qkv_ps", 3, "PSUM")  # outlives the LN psum pools
    ps_st = pp.open("ln_st", 1, "PSUM")

    def k_bundle(j, cc, psum_pool, nm="qkv", act_copy=False):
        cs = slice(cc * TCH, (cc + 1) * TCH)
        ps = psum_pool.tile([P, TCH], F32, name=nm)
        for cp in range(CP):
            nc.tensor.matmul(
                ps[:], wk8s[:, j, cp, :, :], xn8[cp][:, :, cs],
                start=(cp == 0), stop=(cp == CP - 1), perf_mode=PM.DoubleRow,
            )
        if act_copy:
            nc.scalar.activation(
                kt[j][:, cs], ps[:], AF.Identity, bias=gbt["kb"][:, j : j + 1]
            )
        else:
            nc.vector.tensor_scalar_add(kt[j][:, cs], ps[:], gbt["kb"][:, j : j + 1])

    def v_bundle(t, dc, psum_pool, nm="qkvv"):
        gs = slice(dc * 512, (dc + 1) * 512)
        hs = slice(dc * 8, (dc + 1) * 8)
        ps = psum_pool.tile([P, 8, D], F32, name=nm)
        for cp in range(CP):
            nc.tensor.matmul(
                ps[:, :, :], xn8[cp][:, :, t * P : (t + 1) * P], wv8s[:, cp, :, gs],
                start=(cp == 0), stop=(cp == CP - 1), perf_mode=PM.DoubleRow,
            )
        # GPSIMD cannot read PSUM, so these copies stay on DVE
        nc.vector.tensor_tensor(
            vsb65[t // 2][:, t % 2, hs, 0:D], ps[:, :, :], vbT[:, hs, :], OP.add
        )

    def q_bundle(j, psum_pool=None, nm="qkv", act_copy=True):
        ps = (psum_pool or ps_qkv).tile([P, Tq], F32, name=nm)
        for cp in range(CP):
            nc.tensor.matmul(
                ps[:], wq8s[:, j, cp, :, :], xn8[cp][:, :, T - Tq :],
                start=(cp == 0), stop=(cp == CP - 1), perf_mode=PM.DoubleRow,
            )
        if act_copy:
            nc.scalar.activation(
                qt[j][:], ps[:], AF.Identity, bias=gbt["qb"][:, j : j + 1]
            )
        else:
            nc.vector.tensor_scalar_add(qt[j][:], ps[:], gbt["qb"][:, j : j + 1])

    # fused per-column-chunk LN1: stats (squares on DVE) -> per-cc rstd ->
    # fp8 xn8 writes -> QKV bundles for that chunk, pipelined across chunks
    for cc in range(NQC):
        cs = slice(cc * TCH, (cc + 1) * TCH)
        s1 = ps_st.tile([1, TCH], F32, name="s1")
        s2 = ps_st.tile([1, TCH], F32, name="s2")
        for e in range(KE):
            x2 = p_tmp.tile([P, TCH], BF16, name="x2bf")
            if e % 2 == 0:
                nc.vector.tensor_tensor(x2[:], xtb[e][:, cs], xtb[e][:, cs], OP.mult)
            else:
                nc.scalar.square(x2[:], xtb[e][:, cs])
            nc.tensor.matmul(
                s1[:], ones_bf[:], xtb[e][:, cs], start=(e == 0), stop=(e == KE - 1)
            )
            nc.tensor.matmul(s2[:], ones_bf[:], x2[:], start=(e == 0), stop=(e == KE - 1))
        mu_bf = p_rows.tile([1, TCH], BF16, name="mu_bf")
        nc.vector.tensor_scalar_mul(mu_bf[:], s1[:], 1.0 / E)
        ve = p_tmp.tile([1, TCH], F32, name="ve")
        nc.vector.tensor_scalar(ve[:], s2[:], 1.0 / E, c.eps, OP.mult, OP.add)
        mu2 = p_tmp.tile([1, TCH], F32, name="mu2")
        nc.vector.tensor_tensor(mu2[:], mu_bf[:], mu_bf[:], OP.mult)
        vee = p_rows.tile([1, TCH], F32, name="vee")
        nc.vector.tensor_tensor(vee[:], ve[:], mu2[:], OP.subtract)
        iv = p_rows.tile([1, TCH], F32, name="iv")
        nc.vector.reciprocal(iv[:], vee[:])
        rstd = p_rows.tile([1, TCH], BF16, name="rstd")
        nc.scalar.activation(rstd[:], iv[:], AF.Sqrt)
        mub_sb = p_stg.tile([P, TCH], BF16, name="mub_sb")
        nc.gpsimd.partition_broadcast(mub_sb[:], mu_bf[:], channels=P)
        rsb_sb = p_stg.tile([P, TCH], BF16, name="rsb_sb")
        nc.gpsimd.partition_broadcast(rsb_sb[:], rstd[:], channels=P)
        for e in range(KE):
            t1 = p_tmp.tile([P, TCH], BF16, name="t1")
            nc.vector.tensor_tensor(t1[:], xtb[e][:, cs], mub_sb[:], OP.subtract)
            # split the fp8 writes between DVE and GPSIMD to halve the
            # DVE-bound stretch that gates the QKV projections
            eng = nc.vector if e % 2 == 0 else nc.gpsimd
            eng.tensor_tensor(xn8[e // 2][:, e % 2, cs], t1[:], rsb_sb[:], OP.mult)
        # interleave QKV work that only needs this col-chunk of xn8
        if cc == NQC - 1:
            q_bundle(0)
            q_bundle(1)
        k_bundle(0, cc, ps_qkv, act_copy=True)
        k_bundle(1, cc, ps_qkv, act_copy=True)

    pp.close("ln_stg", "ln_rows", "ln_tmp", "xtp", "ln_st")

    # V weights + value tiles (opened after xtb frees its SBUF)
    p_wv8 = pp.open("wv8p", 1)
    wv8s = p_wv8.tile([P, CP, 2, E], FP8, name="wv8s")
    nc.sync.dma_start(wv8s[:], d["wv8"])
    p_vs = pp.open("vsp", 1)
    # V with a ones column appended per head: attn@V row 64 = softmax row-sum
    vsb65 = [
        p_vs.tile([P, 2, H, D + 1], FP8, name=f"vsb65_{tp}") for tp in range(TK // 2)
    ]
    for tp in range(TK // 2):
        nc.vector.memset(vsb65[tp][:, :, :, D : D + 1], 1.0)

    # W1 prefetch into the SBUF freed by xtb (runs during attention); the
    # last chunks stream during MLP1 itself.
    W1PF = 12
    p_w1 = pp.open("w1s", 1, side="right")
    w1f = [p_w1.tile([P, KE, P], BF16, name=f"w1cb{f}") for f in range(W1PF)]
    for f in range(W1PF):
        nc.sync.dma_start(
            w1f[f][:],
            d["w1"].rearrange("(e p) m -> p e m", p=P)[:, :, f * P : (f + 1) * P],
        )
    p_a8 = pp.open("aop8", 1, side="right")
    aop8 = [p_a8.tile([P, 2, Tq], FP8, name=f"aop8_{pq}") for pq in range(CP)]
    p_nb = pp.open("nrm", 3)
    # seed only the first t-pair of V; the rest interleave into group 0
    for t in range(2):
        v_bundle(t, 0, ps_qkv)
    pp.close("qkv_ps")

    # ======================================================================
    # Phase 2: attention (2-head groups; bf16 scores + identity-matmul band
    # bias; fp8 probabilities; DoubleRow attn@V over t-pairs with a ones
    # column so psum row 64 accumulates the softmax row-sums; next-group K
    # projections + V dims 512: interleaved into the stream)
    # ======================================================================
    NG = HP  # one head-pair per group

    p_pr = pp.open("probs", 6)
    p_st2 = pp.open("rstage", 3)
    ps_k2 = pp.open("k2_ps", 1, "PSUM")
    ps_sc = pp.open("sc_ps", 2, "PSUM")
    ps_o = pp.open("o_ps", 1, "PSUM")

    def interleave(g, tp):
        if g == 0:
            # rest of V dims 0-511, one t-pair ahead of its attnV use
            if tp < 7:
                v_bundle(2 + 2 * tp, 0, ps_k2, nm="kv_psv")
                v_bundle(3 + 2 * tp, 0, ps_k2, nm="kv_psv")
            else:
                k_bundle(2, 0, ps_k2, nm="kv_ps")
                k_bundle(2, 1, ps_k2, nm="kv_ps")
        elif g in (1, 2):
            # V dims 512-1023 (heads 8-15, used from group 4 on)
            v_bundle(8 * (g - 1) + tp, 1, ps_k2, nm="kv_psv")
            if tp % 2 == 1:
                k_bundle(g + 2, tp // 2, ps_k2, nm="kv_ps")
            elif g == 1:
                # finish kt[2], then deferred Q projections (DVE copies so
                # the exp-pacing ACT engine stays untouched)
                if tp == 0:
                    k_bundle(2, 2, ps_k2, nm="kv_ps")
                elif tp == 2:
                    k_bundle(2, 3, ps_k2, nm="kv_ps")
                else:
                    q_bundle(2 + (tp - 4) // 2, ps_k2, nm="kv_ps", act_copy=False)
            else:
                q_bundle(4 + tp // 2, ps_k2, nm="kv_ps", act_copy=False)
        elif g < NG - 2 and tp % 2 == 0:
            k_bundle(g + 2, tp // 2, ps_k2, nm="kv_ps")

    for g in range(NG):
        j = g
        oh65 = [ps_o.tile([D + 1, Tq], F32, name=f"oh65_{s}") for s in (0, 1)]
        for tp in range(TK // 2):
            prt = p_pr.tile([P, 2, 2, Tq], FP8, name="prt")
            for ti in (0, 1):
                t = 2 * tp + ti
                bt = t - (TK - NB)
                ss = ps_sc.tile([P, 2, Tq], F32, name="ss")
                for s in (0, 1):
                    nc.tensor.matmul(
                        ss[:, s, :],
                        kt[j][s * 64 : (s + 1) * 64, t * P : (t + 1) * P],
                        qt[j][s * 64 : (s + 1) * 64, :],
                        start=True, stop=(bt < 0),
                        tile_position=(s * 64, 0),
                    )
                if bt >= 0:
                    for s in (0, 1):
                        nc.tensor.matmul(
                            ss[:, s, :], ident[:], negband[:, bt, :],
                            start=False, stop=True,
                            tile_position=(0, 0), skip_group_check=True,
                        )
                nc.scalar.activation(
                    prt[:, :, ti, :], ss[:, :, :], AF.Exp,
                    bias=colmask[:, t : t + 1], scale=SCL,
                )
            for s in (0, 1):
                h = 2 * j + s
                nc.tensor.matmul(
                    oh65[s][:, :],
                    vsb65[tp][:, :, h, :],
                    prt[:, s, :, :],
                    start=(tp == 0), stop=(tp == TK // 2 - 1),
                    tile_position=(0, 0),
                    skip_group_check=True, perf_mode=PM.DoubleRow,
                )
            interleave(g, tp)
        # Row 64 of the attn@V psum holds the softmax row-sums: invert at
        # partition 0, broadcast on GPSIMD, multiply straight out of PSUM
        # into fp8 aop8 (head 2j+1's half lands via a partition-shift DMA).
        for s in (0, 1):
            irsr = p_st2.tile([1, Tq], F32, name=f"irs_r{s}")
            nc.vector.reciprocal(irsr[:], oh65[s][D : D + 1, :])
            irsb = p_st2.tile([1, Tq], BF16, name=f"irs_b{s}")
            nc.vector.tensor_copy(irsb[:], irsr[:])
            nbs = p_nb.tile([64, Tq], BF16, name=f"nbs{s}")
            nc.gpsimd.partition_broadcast(nbs[:], irsb[:], channels=64)
            if s == 0:
                nc.vector.tensor_tensor(
                    aop8[j // 2][0:64, j % 2, :], oh65[0][0:D, :], nbs[:], OP.mult
                )
            else:
                s1m = p_st2.tile([D, Tq], FP8, name="s1m")
                nc.vector.tensor_tensor(s1m[:], oh65[1][0:D, :], nbs[:], OP.mult)
                nc.gpsimd.dma_start(aop8[j // 2][64:128, j % 2, :], s1m[:])
        if g == 3:
            # late loads overlapped with the second half of attention
            for e in range(KE):
                nc.sync.dma_start(xq32[e][:], d["xq32"][e * P : (e + 1) * P, :])
            nc.sync.dma_start(wo8s[:], d["wo8"])

    pp.close("rstage", "probs", "nrm")
    pp.close("o_ps", "sc_ps", "k2_ps")
    pp.close("vsp", "wv8p", "wq8p", "wk8p", "ktp", "qtp", "xn8p")

    # ======================================================================
    # Phase 3: out-projection + residual -> xres, with LayerNorm2 stats
    # interleaved per chunk; then xn2
    # ======================================================================
    p_xr = pp.open("xrp", 1, side="right")
    p_x2 = pp.open("xn2p", 1, side="right")
    p_tmp = pp.open("ln2_tmp", 2)
    p_rows = pp.open("ln2_rows", 1)
    ps_st = pp.open("ln2_st", 1, "PSUM")
    ps_ao = pp.open("ao_ps", 2, "PSUM")

    xres = [p_xr.tile([P, Tq], F32, name=f"xres{e}") for e in range(KE)]
    xn2 = [p_x2.tile([P, Tq], BF16, name=f"xn2{e}") for e in range(KE)]

    s1 = ps_st.tile([1, Tq], F32, name="s1b")
    s2 = ps_st.tile([1, Tq], F32, name="s2b")
    for e in range(KE):
        ps = ps_ao.tile([P, Tq], F32, name="aops")
        for pq in range(CP):
            nc.tensor.matmul(
                ps[:], wo8s[:, e, pq, :, :], aop8[pq][:],
                start=(pq == 0), stop=(pq == CP - 1), perf_mode=PM.DoubleRow,
            )
        nc.vector.scalar_tensor_tensor(
            xres[e][:], ps[:], gbt["boc"][:, e : e + 1], xq32[e][:], OP.add, OP.add
        )
        xbf = p_tmp.tile([P, Tq], BF16, name="xbf2")
        nc.vector.tensor_copy(xbf[:], xres[e][:])
        x2 = p_tmp.tile([P, Tq], BF16, name="x2bf2")
        nc.scalar.square(x2[:], xres[e][:])
        nc.tensor.matmul(s1[:], ones_bf[:], xbf[:], start=(e == 0), stop=(e == KE - 1))
        nc.tensor.matmul(s2[:], ones_bf[:], x2[:], start=(e == 0), stop=(e == KE - 1))
    pp.close("ao_ps")
    mu2b = p_rows.tile([1, Tq], BF16, name="mu_2")
    nc.vector.tensor_scalar_mul(mu2b[:], s1[:], 1.0 / E)
    ve = p_rows.tile([1, Tq], F32, name="ve_2")
    nc.vector.tensor_scalar(ve[:], s2[:], 1.0 / E, c.eps, OP.mult, OP.add)
    mu22 = p_rows.tile([1, Tq], F32, name="mu2_2")
    nc.vector.tensor_tensor(mu22[:], mu2b[:], mu2b[:], OP.mult)
    vee = p_rows.tile([1, Tq], F32, name="vee_2")
    nc.vector.tensor_tensor(vee[:], ve[:], mu22[:], OP.subtract)
    iv2 = p_rows.tile([1, Tq], F32, name="iv_2")
    nc.vector.reciprocal(iv2[:], vee[:])
    rstd = p_rows.tile([1, Tq], BF16, name="rstd_2")
    nc.scalar.activation(rstd[:], iv2[:], AF.Sqrt)
    mub_sb = p_rows.tile([P, Tq], BF16, name="mub2_sb")
    nc.gpsimd.partition_broadcast(mub_sb[:], mu2b[:], channels=P)
    rsb_sb = p_rows.tile([P, Tq], BF16, name="rsb2_sb")
    nc.gpsimd.partition_broadcast(rsb_sb[:], rstd[:], channels=P)
    for e in range(KE):
        t1 = p_tmp.tile([P, Tq], BF16, name="t1b")
        nc.vector.tensor_tensor(t1[:], xres[e][:], mub_sb[:], OP.subtract)
        eng = nc.vector if e % 2 == 0 else nc.gpsimd
        eng.tensor_tensor(xn2[e][:], t1[:], rsb_sb[:], OP.mult)
    pp.close("ln2_rows", "ln2_tmp", "ln2_st")

    # ======================================================================
    # Phase 4+5: MLP (bf16; layer 1 streamed with first-half layer 2)
    # ======================================================================
    EH = min(KE, 6)  # h2 chunks accumulated under MLP1 (PSUM: 6 + 2 h1 bufs)
    p_h1 = pp.open("h1p", 1, side="right")
    p_w2 = pp.open("w2s", 6)
    p_out = pp.open("outp", 2)
    ps_h1 = pp.open("h1_ps", 2, "PSUM")
    ps_h2a = pp.open("h2a_ps", 1, "PSUM")

    p_w1b = pp.open("w1b", 4)
    p_w2b = pp.open("w2b", 1)
    # single prefetch of the W2 columns the second MLP2 pass needs, issued
    # before MLP1 so the tail never waits on DMA issue
    w2ball = p_w2b.tile([P, KF, (KE - EH) * P], BF16, name="w2ball")
    nc.sync.dma_start(
        w2ball[:],
        d["w2"].rearrange("(f p) m -> p f m", p=P)[:, :, EH * P : KE * P],
    )
    h1 = [p_h1.tile([P, Tq], BF16, name=f"h1{f}") for f in range(KF)]
    h2a = [ps_h2a.tile([P, Tq], F32, name=f"h2a{e}") for e in range(EH)]
    for f in range(KF):
        if f < W1PF:
            w1c = w1f[f]
        else:
            w1c = p_w1b.tile([P, KE, P], BF16, name="w1cs")
            nc.sync.dma_start(
                w1c[:],
                d["w1"].rearrange("(e p) m -> p e m", p=P)[:, :, f * P : (f + 1) * P],
            )
        ps = ps_h1.tile([P, Tq], F32, name="h1ps")
        for e in range(KE):
            nc.tensor.matmul(
                ps[:], w1c[:, e, :], xn2[e][:], start=(e == 0), stop=(e == KE - 1)
            )
        nc.scalar.activation(
            h1[f][:], ps[:], AF.Relu, bias=gbt["mb1"][:, f : f + 1], scale=1.0
        )
        w2f = p_w2.tile([P, EH * P], BF16, name="w2sa")
        nc.sync.dma_start(w2f[:], d["w2"][f * P : (f + 1) * P, 0 : EH * P])
        for e in range(EH):
            nc.tensor.matmul(
                h2a[e][:], w2f[:, e * P : (e + 1) * P], h1[f][:],
                start=(f == 0), stop=(f == KF - 1),
            )
    for e in range(EH):
        of = p_out.tile([P, Tq], F32, name="outf")
        nc.vector.scalar_tensor_tensor(
            of[:], h2a[e][:], gbt["mb2"][:, e : e + 1], xres[e][:], OP.add, OP.add
        )
        nc.sync.dma_start(d["out_t"][e * P : (e + 1) * P, :], of[:])
    pp.close("h2a_ps", "h1_ps")

    if EH < KE:
        ps_h2b = pp.open("h2b_ps", 1, "PSUM")
        h2b = [ps_h2b.tile([P, Tq], F32, name=f"h2b{e}") for e in range(KE - EH)]
        for f in range(KF):
            for i in range(KE - EH):
                nc.tensor.matmul(
                    h2b[i][:], w2ball[:, f, i * P : (i + 1) * P], h1[f][:],
                    start=(f == 0), stop=(f == KF - 1),
                )
        for i, e in enumerate(range(EH, KE)):
            of = p_out.tile([P, Tq], F32, name="outf")
            nc.vector.scalar_tensor_tensor(
                of[:], h2b[i][:], gbt["mb2"][:, e : e + 1], xres[e][:], OP.add, OP.add
            )
            nc.sync.dma_start(d["out_t"][e * P : (e + 1) * P, :], of[:])

    pp.close_all()


def build_program(c: Cfg = CFG, reps: int = 1):
    c.check()
    nc = bacc.Bacc(
        "TRN2",
        target_bir_lowering=False,
        debug=False,
        enable_asserts=False,
        num_devices=c.NC,
    )
    CP = c.KE // 2
    d = {}
    d["xtb"] = nc.dram_tensor("xtb", [c.E, c.T], BF16, kind="ExternalInput").ap()
    d["xq32"] = nc.dram_tensor("xq32", [c.E, c.Tq], F32, kind="ExternalInput").ap()
    d["wq8"] = nc.dram_tensor("wq8", [P, c.HP, CP, 2, P], FP8, kind="ExternalInput").ap()
    d["wk8"] = nc.dram_tensor("wk8", [P, c.HP, CP, 2, P], FP8, kind="ExternalInput").ap()
    d["wv8"] = nc.dram_tensor("wv8", [P, CP, 2, c.E], FP8, kind="ExternalInput").ap()
    d["wo8"] = nc.dram_tensor("wo8", [P, c.KE, CP, 2, P], FP8, kind="ExternalInput").ap()
    d["w1"] = nc.dram_tensor("w1", [c.E, c.F], BF16, kind="ExternalInput").ap()
    d["w2"] = nc.dram_tensor("w2", [c.F, c.E], BF16, kind="ExternalInput").ap()
    for nm, cols in [
        ("qb", c.KE), ("kb", c.KE), ("boc", c.KE), ("mb1", c.KF), ("mb2", c.KE),
    ]:
        d[nm] = nc.dram_tensor(nm, [P, cols], F32, kind="ExternalInput").ap()
    d["vbrow"] = nc.dram_tensor("vbrow", [1, c.E], BF16, kind="ExternalInput").ap()
    d["colmask"] = nc.dram_tensor("colmask", [P, c.TK], F32, kind="ExternalInput").ap()
    d["ident"] = nc.dram_tensor("ident", [P, P], BF16, kind="ExternalInput").ap()
    d["negband"] = nc.dram_tensor(
        "negband", [P, c.NB, c.Tq], BF16, kind="ExternalInput"
    ).ap()
    d["out_t"] = nc.dram_tensor("out_t", [c.E, c.Tq], F32, kind="ExternalOutput").ap()

    with tile.TileContext(nc) as tc:
        _emit(tc, c, d, reps=reps)
    nc.compile()
    return nc


# --------------------------------------------------------------------------
# host side
# --------------------------------------------------------------------------
def shard_inputs(inputs, c: Cfg = CFG):
    CP = c.KE // 2
    x = np.ascontiguousarray(np.asarray(inputs["x"], np.float32))
    f32 = lambda a: np.asarray(a, np.float32)

    g1 = f32(inputs["ln1_g"])
    b1v = f32(inputs["ln1_b"])
    g2 = f32(inputs["ln2_g"])
    b2v = f32(inputs["ln2_b"])
    Wq = g1[:, None] * f32(inputs["Wq"])
    Wk = g1[:, None] * f32(inputs["Wk"])
    Wv = g1[:, None] * f32(inputs["Wv"])
    qb = b1v @ f32(inputs["Wq"])
    kb = b1v @ f32(inputs["Wk"])
    vb = b1v @ f32(inputs["Wv"])
    W1 = g2[:, None] * f32(inputs["W1"])
    mb1 = f32(inputs["b1"]) + b2v @ f32(inputs["W1"])

    def pack_qk(W):  # [E, E] -> [p, j, cp, i, m]
        A = W.reshape(CP, 2, P, c.HP, P).transpose(2, 3, 0, 1, 4)
        return np.ascontiguousarray(A).astype(NPF8)

    def pack_v(W):  # [E, E] -> [p, cp, i, d]
        A = W.reshape(CP, 2, P, c.E).transpose(2, 0, 1, 3)
        return np.ascontiguousarray(A).astype(NPF8)

    def pack_o(W):  # [E, E] -> [p, e, pp, i, m]
        A = W.reshape(CP, 2, P, c.KE, P).transpose(2, 3, 0, 1, 4)
        return np.ascontiguousarray(A).astype(NPF8)

    chunks = lambda v, k: np.ascontiguousarray(f32(v).reshape(k, P).T)
    com = {
        "wq8": pack_qk(Wq),
        "wk8": pack_qk(Wk),
        "wv8": pack_v(Wv),
        "wo8": pack_o(f32(inputs["Wo"])),
        "w1": np.ascontiguousarray(W1).astype(NPBF16),
        "w2": np.ascontiguousarray(f32(inputs["W2"])).astype(NPBF16),
        "qb": chunks(qb, c.KE),
        "kb": chunks(kb, c.KE),
        "boc": chunks(inputs["bo"], c.KE),
        "mb1": chunks(mb1, c.KF),
        "mb2": chunks(inputs["b2"], c.KE),
        "vbrow": np.ascontiguousarray(vb.reshape(1, c.E)).astype(NPBF16),
        "ident": np.eye(P, dtype=np.float32).astype(NPBF16),
    }

    p_idx = np.arange(P)[:, None]
    tq_idx = np.arange(c.Tq)[None, :]
    nb = np.zeros((P, c.NB, c.Tq), np.float32)
    for jb in range(c.NB):
        nb[:, jb, :] = np.where(tq_idx >= (jb * P + p_idx), 0.0, NEG)
    com["negband"] = nb.astype(NPBF16)

    slot = np.arange(c.T)
    maps = []
    for core in range(c.NC):
        b, qi = core // c.CPB, core % c.CPB
        qoff = qi * c.Tq
        pad = c.T - qoff - c.Tq
        ctx = np.zeros((c.T, c.E), np.float32)
        ctx[pad:, :] = x[b, : qoff + c.Tq, :]
        colmask = np.ascontiguousarray(
            np.where(slot.reshape(c.TK, P).T < pad, NEG, ESHIFT).astype(np.float32)
        )
        m = dict(com)
        m["xtb"] = np.ascontiguousarray(ctx.T).astype(NPBF16)
        m["xq32"] = np.ascontiguousarray(x[b, qoff : qoff + c.Tq, :].T)
        m["colmask"] = colmask
        maps.append(m)
    return maps


def assemble(results, c: Cfg = CFG):
    out = np.empty((c.B, c.T, c.E), np.float32)
    for core in range(c.NC):
        b, qi = core // c.CPB, core % c.CPB
        out[b, qi * c.Tq : (qi + 1) * c.Tq, :] = results[core]["out_t"].T
    return out


_NC_CACHE = {}


def _get_nc(c: Cfg = CFG):
    if c not in _NC_CACHE:
        _NC_CACHE[c] = build_program(c)
    return _NC_CACHE[c]


LAST_RESULT = None


def kernel(**inputs):
    global LAST_RESULT
    c = CFG
    nc = _get_nc(c)
    maps = shard_inputs(inputs, c)
    res = bass_utils.run_bass_kernel_spmd(nc, maps, core_ids=list(range(c.NC)))
    LAST_RESULT = res
    return assemble(res.results, c)
